# revision 8
# baseline (speedup 1.0000x reference)
"""ConvNCF Trainium2 kernel (8 NeuronCores, data-parallel over batch).

Sharding: batch 4096 -> 8 cores x 512 samples.  Per core the device batch is
1024 rows ([512 pos | 512 neg]); rows are split into 4 partition groups
g = n // 256 of 32 channels each, so every conv layer runs as 4 concurrent
32x32 PE tiles at tile_position (32g, 32g) ("diagonal" tiling).

The host performs only the embedding row lookup (the device runtime's
indirect-DMA gather scrambles multi-row-per-partition transfers, verified
empirically) and ships the 2x128KB of gathered fp16 rows per core; everything
else runs on device:

1. R-permute matmuls expand the 4 gathered row-groups into the conv1 im2col
   u/v factor layout upat/vpat[32g + 8a + 2b + d, (s, p)] = u[n, 2p+a-1],
   using per-matmul shifted stride-2 windows for the tap offset (edge taps
   simply skip the out-of-range output column; PSUM accumulation of the
   one-hot R blocks assembles all rows).
2. A broadcast tensor_tensor produces conv1 outer-product patches
   patches[pi, (s,p,q)] = upat[pi,(s,p)] * vpat[pi,(s,q)], so one K=32 matmul
   per 512 columns evaluates all 16 conv1 taps (host halves w1 for the dup).
3. conv2..6: 16 accumulating K=32 tap-matmuls per group over zero-padded fp16
   SBUF tiles with strided APs; fp32 PSUM; ScalarE fuses bias+relu while
   evacuating into the next layer's padded interior.
4. Head: K=32 matmul + fused sigmoid per group, fp32 out.
"""

import os

import numpy as np

B, D, NFM = 4096, 64, 32
N_CORES = 8
NB = B // N_CORES          # 512 samples per core
NDEV = 2 * NB              # 1024 device rows (pos branch then neg branch)
NG = NDEV // 4             # 256 rows per partition group
N_TILES = 32
ST = NG // N_TILES         # 8 slots per group per tile

# layer -> padded input side;  layer -> output side
PAD = {2: 34, 3: 18, 4: 10, 5: 6, 6: 4}
OUT = {1: 32, 2: 16, 3: 8, 4: 4, 5: 2, 6: 1}

# shift windows: tap-row a covers out p in [lo, hi), reading col 2p + (a-1)
WIN = []
for _a in range(4):
    _sig = _a - 1
    _lo = 1 if _sig < 0 else 0
    _hi = 32 if 2 * 31 + _sig <= 63 else 31
    WIN.append((_lo, _hi, 2 * _lo + _sig))


def _build_program():
    import concourse.bacc as bacc
    import concourse.tile as tile
    from concourse import mybir

    F16 = mybir.dt.float16
    F32 = mybir.dt.float32
    AF = mybir.ActivationFunctionType

    nc = bacc.Bacc("TRN2", target_bir_lowering=False, name="convncf")

    ug_t = nc.dram_tensor("ug", [4, NG * 64], F16, kind="ExternalInput")
    vg_t = nc.dram_tensor("vg", [4, NG * 64], F16, kind="ExternalInput")
    rmat_t = nc.dram_tensor("rmat", [32, 8 * 128], F16, kind="ExternalInput")
    w1d_t = nc.dram_tensor("w1d", [128, NFM], F16, kind="ExternalInput")
    wrest_t = nc.dram_tensor("wrest", [128, 5 * 16 * NFM], F16, kind="ExternalInput")
    wpred_t = nc.dram_tensor("wpred", [128, 1], F16, kind="ExternalInput")
    bias_t = nc.dram_tensor("biases", [128, 8], F32, kind="ExternalInput")
    out_t = nc.dram_tensor("out", [4, NG], F32, kind="ExternalOutput")

    with tile.TileContext(nc) as tc:
        with (
            tc.tile_pool(name="const", bufs=1) as constp,
            tc.tile_pool(name="glob", bufs=1) as globp,
            tc.tile_pool(name="work", bufs=2) as workp,
            tc.tile_pool(name="ps1", bufs=2, space="PSUM") as ps1p,
            tc.tile_pool(name="ps2", bufs=2, space="PSUM") as ps2p,
            tc.tile_pool(name="ps3", bufs=2, space="PSUM") as ps3p,
        ):
            rmat = constp.tile([32, 8 * 128], F16, name="rmat")
            w1d = constp.tile([128, NFM], F16, name="w1d")
            wrest = constp.tile([128, 5 * 16 * NFM], F16, name="wrest")
            wpred = constp.tile([128, 1], F16, name="wpred")
            biases = constp.tile([128, 8], F32, name="biases")
            upat = globp.tile([128, NG * 32], F16, name="upat")
            vpat = globp.tile([128, NG * 32], F16, name="vpat")
            x5 = globp.tile([128, NG * 36], F16, name="x5")
            x6 = globp.tile([128, NG * 16], F16, name="x6")
            y6 = globp.tile([128, NG], F16, name="y6")
            outsb = globp.tile([128, NG], F32, name="outsb")

            nc.sync.dma_start(rmat[:], rmat_t[:])
            nc.sync.dma_start(w1d[:], w1d_t[:])
            nc.sync.dma_start(wrest[:], wrest_t[:])
            nc.sync.dma_start(wpred[:], wpred_t[:])
            nc.sync.dma_start(biases[:], bias_t[:])
            nc.gpsimd.memset(x5[:], 0.0)
            nc.gpsimd.memset(x6[:], 0.0)

            # ---- R-permute into upat/vpat (staging pool freed afterwards) ----
            with tc.tile_pool(name="pre", bufs=1) as prep:
                stg = prep.tile([128, NG * 64], F16, name="stg")
                nc.gpsimd.memset(stg[:], 0.0)
                st3 = stg[:].rearrange("c (s e) -> c s e", e=64)
                SCH = 16  # slots per psum chunk -> 512 cols
                order = [1, 0, 2, 3]
                for tbl in range(2):
                    nc.sync.dma_start(stg[0:4, :], (ug_t if tbl == 0 else vg_t)[:])
                    src3 = st3
                    dstp = upat if tbl == 0 else vpat
                    for ch in range(NG // SCH):
                        s0 = ch * SCH
                        ps = ps2p.tile([128, 512], F32, tag="ps2", name="psr")
                        for i, t in enumerate(order):
                            lo, hi, o = WIN[t]
                            rhs = src3[0:32, s0 : s0 + SCH, o : o + 2 * (hi - lo) - 1 : 2]
                            dst = ps[:].rearrange("c (s q) -> c s q", q=32)[:, :, lo:hi]
                            nc.tensor.matmul(
                                dst,
                                rmat[:, 128 * (4 * tbl + t) : 128 * (4 * tbl + t) + 128],
                                rhs,
                                start=(i == 0),
                                stop=(i == 3),
                            )
                        nc.scalar.activation(
                            dstp[:, s0 * 32 : (s0 + SCH) * 32], ps[:], AF.Copy
                        )

            upat3 = upat[:].rearrange("c (s q) -> c s q", q=32)
            vpat3 = vpat[:].rearrange("c (s q) -> c s q", q=32)

            def w_l(layer, t):  # layer 2..6, tap t = 4a+b
                c0 = ((layer - 2) * 16 + t) * NFM
                return wrest[:, c0 : c0 + NFM]

            def conv_layer(layer, xin, xout, psp, pstag, glob_s0=None):
                """16-tap K=32 conv over ST slots/group; evac bias+relu."""
                side = PAD[layer]
                oside = OUT[layer]
                iarea = side * side
                cols_slot = oside * oside
                total = ST * cols_slot
                chw = min(total, 512)
                slots_ch = max(1, chw // cols_slot)
                nch = (total + chw - 1) // chw
                xi = xin[:].rearrange("c (s i) -> c s i", i=iarea)
                opad = PAD.get(layer + 1)
                for ch in range(nch):
                    sa = ch * slots_ch
                    ps = psp.tile([128, chw], F32, tag=pstag, name="psc")
                    for t in range(16):
                        a, b = t // 4, t % 4
                        for g in range(4):
                            rhs = xi[
                                32 * g : 32 * g + 32, sa : sa + slots_ch, :
                            ].rearrange("c s (p q) -> c s p q", p=side)[
                                :,
                                :,
                                a : a + 2 * oside - 1 : 2,
                                b : b + 2 * oside - 1 : 2,
                            ]
                            nc.tensor.matmul(
                                ps[32 * g : 32 * g + 32, :],
                                w_l(layer, t)[32 * g : 32 * g + 32, :],
                                rhs,
                                start=(t == 0),
                                stop=(t == 15),
                                tile_position=(32 * g, 32 * g),
                            )
                    base = (glob_s0 + sa) if glob_s0 is not None else sa
                    if opad:
                        xo = xout[:].rearrange("c (s i) -> c s i", i=opad * opad)
                        dst = xo[:, base : base + slots_ch, :].rearrange(
                            "c s (p q) -> c s p q", p=opad
                        )[:, :, 1 : 1 + oside, 1 : 1 + oside]
                    else:
                        dst = xout[:, base : base + slots_ch]
                    nc.scalar.activation(
                        dst,
                        ps[:].rearrange("c (s p q) -> c s p q", s=slots_ch, p=oside),
                        AF.Relu,
                        bias=biases[:, layer - 1 : layer],
                    )

            # ---------------- tiled conv1..conv4 ----------------
            for ti in range(N_TILES):
                s0 = ti * ST
                patches = workp.tile(
                    [128, ST * 1024], F16, tag="patches", name="patches", bufs=1
                )
                x2 = workp.tile([128, ST * 34 * 34], F16, tag="x2", name="x2")
                x3 = workp.tile([128, ST * 18 * 18], F16, tag="x3", name="x3", bufs=1)
                x4 = workp.tile([128, ST * 10 * 10], F16, tag="x4", name="x4", bufs=1)

                pat4 = patches[:].rearrange("c (s p q) -> c s p q", p=32, q=32)
                u_in = upat3[:, s0 : s0 + ST, :].unsqueeze(3).broadcast_to(
                    [128, ST, 32, 32]
                )
                v_in = vpat3[:, s0 : s0 + ST, :].unsqueeze(2).broadcast_to(
                    [128, ST, 32, 32]
                )
                nc.vector.tensor_tensor(pat4, u_in, v_in, mybir.AluOpType.mult)

                # conv1: one K=32 matmul per 512 cols covers all 16 taps
                for s in range(ST):
                    ps = ps1p.tile([128, 1024], F32, tag="ps1", name="ps1t")
                    for half in range(2):
                        for g in range(4):
                            nc.tensor.matmul(
                                ps[
                                    32 * g : 32 * g + 32,
                                    512 * half : 512 * (half + 1),
                                ],
                                w1d[32 * g : 32 * g + 32, :],
                                patches[
                                    32 * g : 32 * g + 32,
                                    s * 1024
                                    + 512 * half : s * 1024
                                    + 512 * (half + 1),
                                ],
                                start=True,
                                stop=True,
                                tile_position=(32 * g, 32 * g),
                            )
                    dst = x2[:].rearrange("c (s i) -> c s i", i=34 * 34)[
                        :, s : s + 1, :
                    ].rearrange("c s (p q) -> c s p q", p=34)[:, :, 1:33, 1:33]
                    nc.scalar.activation(
                        dst,
                        ps[:].rearrange("c (s p q) -> c s p q", s=1, p=32),
                        AF.Relu,
                        bias=biases[:, 0:1],
                    )

                # zero padded borders of this tile's buffers
                for x, side in ((x2, 34), (x3, 18), (x4, 10)):
                    xr = x[:].rearrange("c (s p q) -> c s p q", s=ST, p=side)
                    nc.gpsimd.memset(xr[:, :, 0 : side : side - 1, :], 0.0)
                    nc.gpsimd.memset(
                        xr[:, :, 1 : side - 1, 0 : side : side - 1], 0.0
                    )

                conv_layer(2, x2, x3, ps1p, "ps1")
                conv_layer(3, x3, x4, ps2p, "ps2")
                conv_layer(4, x4, x5, ps3p, "ps3", glob_s0=s0)

            # ---------------- conv5 (global) ----------------
            x5r = x5[:].rearrange("c (s i) -> c s i", i=36)
            for half in range(2):
                sbase = half * 128
                ps = ps2p.tile([128, 512], F32, tag="ps2", name="ps5")
                for t in range(16):
                    a, b = t // 4, t % 4
                    for g in range(4):
                        rhs = x5r[
                            32 * g : 32 * g + 32, sbase : sbase + 128, :
                        ].rearrange("c s (p q) -> c s p q", p=6)[
                            :, :, a : a + 3 : 2, b : b + 3 : 2
                        ]
                        nc.tensor.matmul(
                            ps[32 * g : 32 * g + 32, :],
                            w_l(5, t)[32 * g : 32 * g + 32, :],
                            rhs,
                            start=(t == 0),
                            stop=(t == 15),
                            tile_position=(32 * g, 32 * g),
                        )
                dst = x6[:].rearrange("c (s i) -> c s i", i=16)[
                    :, sbase : sbase + 128, :
                ].rearrange("c s (p q) -> c s p q", p=4)[:, :, 1:3, 1:3]
                nc.scalar.activation(
                    dst,
                    ps[:].rearrange("c (s p q) -> c s p q", p=2, q=2),
                    AF.Relu,
                    bias=biases[:, 4:5],
                )

            # ---------------- conv6 (global) ----------------
            ps6 = ps2p.tile([128, 512], F32, tag="ps2", name="ps6")
            x6r = x6[:].rearrange("c (s i) -> c s i", i=16)
            for t in range(16):
                a, b = t // 4, t % 4
                for g in range(4):
                    rhs = x6r[32 * g : 32 * g + 32, :, 4 * a + b : 4 * a + b + 1]
                    nc.tensor.matmul(
                        ps6[32 * g : 32 * g + 32, 0:NG],
                        w_l(6, t)[32 * g : 32 * g + 32, :],
                        rhs,
                        start=(t == 0),
                        stop=(t == 15),
                        tile_position=(32 * g, 32 * g),
                    )
            nc.scalar.activation(y6[:], ps6[:, 0:NG], AF.Relu, bias=biases[:, 5:6])

            # ---------------- head ----------------
            psh = ps3p.tile([128, 256], F32, tag="ps3", name="psh")
            for g in range(4):
                nc.tensor.matmul(
                    psh[32 * g : 32 * g + 1, 0:NG],
                    wpred[32 * g : 32 * g + 32, :],
                    y6[32 * g : 32 * g + 32, :],
                    start=True,
                    stop=True,
                    tile_position=(32 * g, 32 * g),
                )
                nc.scalar.activation(
                    outsb[32 * g : 32 * g + 1, :],
                    psh[32 * g : 32 * g + 1, 0:NG],
                    AF.Sigmoid,
                    bias=biases[32 * g : 32 * g + 1, 6:7],
                )
            nc.sync.dma_start(out_t[:], outsb[0:97:32, :])

    nc.compile()
    return nc


def _host_prep(inputs):
    user = np.asarray(inputs["user"]).reshape(-1).astype(np.int64)
    item_pos = np.asarray(inputs["item_pos"]).reshape(-1).astype(np.int64)
    item_neg = np.asarray(inputs["item_neg"]).reshape(-1).astype(np.int64)
    user_w = np.asarray(inputs["user_emb_w"], dtype=np.float32).astype(np.float16)
    item_w = np.asarray(inputs["item_emb_w"], dtype=np.float32).astype(np.float16)
    w1 = np.asarray(inputs["conv1_w"], dtype=np.float32)
    b1 = np.asarray(inputs["conv1_b"], dtype=np.float32)
    wr = np.asarray(inputs["rest_w"], dtype=np.float32)
    br = np.asarray(inputs["rest_b"], dtype=np.float32)
    wp = np.asarray(inputs["pred_w"], dtype=np.float32)
    bp = np.asarray(inputs["pred_b"], dtype=np.float32)

    # R[g, (4*tbl + t)*128 + dst] with dst = 32g + 8a + 2b + d
    rmat = np.zeros((32, 8 * 128), dtype=np.float16)
    for g in range(4):
        for a in range(4):
            for b in range(4):
                for dd in range(2):
                    dst = 32 * g + 8 * a + 2 * b + dd
                    rmat[g, 128 * a + dst] = 1.0          # u-block keyed by a
                    rmat[g, 128 * (4 + b) + dst] = 1.0    # v-block keyed by b
    w1d = np.zeros((128, NFM), dtype=np.float16)
    for g in range(4):
        for a in range(4):
            for b in range(4):
                for dd in range(2):
                    w1d[32 * g + 8 * a + 2 * b + dd, :] = 0.5 * w1[:, 0, a, b]
    wrest = np.zeros((128, 5 * 16 * NFM), dtype=np.float16)
    for L in range(5):
        for a in range(4):
            for b in range(4):
                col0 = (L * 16 + 4 * a + b) * NFM
                blkT = wr[L, :, :, a, b].T  # [cin, cout]
                for g in range(4):
                    wrest[32 * g : 32 * g + 32, col0 : col0 + NFM] = blkT
    wpred = np.zeros((128, 1), dtype=np.float16)
    biases = np.zeros((128, 8), dtype=np.float32)
    for g in range(4):
        wpred[32 * g : 32 * g + 32, 0] = wp[0, :]
        biases[32 * g : 32 * g + 32, 0] = b1
        for L in range(5):
            biases[32 * g : 32 * g + 32, 1 + L] = br[L]
    biases[:, 6] = bp[0]

    in_maps = []
    for c in range(N_CORES):
        u = user[NB * c : NB * c + NB]
        ip = item_pos[NB * c : NB * c + NB]
        ineg = item_neg[NB * c : NB * c + NB]
        ug = np.zeros((4, NG * 64), dtype=np.float16)
        vg = np.zeros((4, NG * 64), dtype=np.float16)
        for g in range(4):
            j0 = NG * (g % 2)
            uids = u[j0 : j0 + NG]
            vsrc = ip if g < 2 else ineg
            vids = vsrc[j0 : j0 + NG]
            ug[g] = user_w[uids].reshape(-1)
            vg[g] = item_w[vids].reshape(-1)
        in_maps.append(
            dict(
                ug=ug,
                vg=vg,
                rmat=rmat,
                w1d=w1d,
                wrest=wrest,
                wpred=wpred,
                biases=biases,
            )
        )
    return in_maps


_CACHED = {}


def kernel_with_stats(**inputs):
    from concourse.bass_utils import run_bass_kernel_spmd

    if "nc" not in _CACHED:
        _CACHED["nc"] = _build_program()
    nc = _CACHED["nc"]
    in_maps = _host_prep(inputs)
    res = run_bass_kernel_spmd(
        nc,
        in_maps,
        core_ids=list(range(N_CORES)),
        trace=bool(int(os.environ.get("CONVNCF_TRACE", "0"))),
    )
    out1 = np.zeros((B, 1), dtype=np.float32)
    out2 = np.zeros((B, 1), dtype=np.float32)
    for c in range(N_CORES):
        o = res.results[c]["out"]  # [4, NG]
        out1[NB * c : NB * c + NB, 0] = o[0:2].reshape(-1)
        out2[NB * c : NB * c + NB, 0] = o[2:4].reshape(-1)
    return (out1, out2), res


def kernel(**inputs):
    out, _ = kernel_with_stats(**inputs)
    return out
